# revision 26
# baseline (speedup 1.0000x reference)
"""Cross-temporal attention Trainium2 (Bass/Tile) kernel.

Problem: two streams x1, x2 of shape [B=4, C=256, H=64, W=64]; tokens are the
H*W=4096 spatial positions. Per batch b and stream s:
    q_s = t_s @ Wq.T + bq ; k_s = t_s @ Wk.T + bk ; v_s = t_s @ Wv.T + bv
    out_s = softmax(q_s @ k_{3-s}.T) @ v_s            (no 1/sqrt(d) scaling)

Sharding: 8 NeuronCores, one (batch, stream) unit per core (4 batches x 2
streams). Fully SPMD — the same program runs on every core, only the input
bindings differ. No collectives.

Per-core layout trick: x[b] is already [C, N] channel-major, which is exactly
the transposed token matrix. All intermediates stay transposed:
    QT = Wq @ X + bq   [C, N]      (PE: lhsT = Wq^T chunks, rhs = X chunks)
    KT = Wk @ Xo + bk  [C, N]
    V  = X^T @ Wv^T + bv  [N, C]   (PE: lhsT = X chunks, rhs = Wv^T)
    ST = KT^T-block @ QT = scores^T  [m, n] blocks   (softmax over m = partitions)
    E  = exp(ST)   (no max subtraction: |logits| < ~40 << 88, fp32-safe)
    U  = accum_m V^T-block @ E  -> [C, n] unnormalized out^T
    D  = column sums of E (ones-matmul replicates to all partitions)
    OT = U / D     [C, N] == x_out[b] flattened. No transposes anywhere.

All matmuls run in float32r (TF32) at 1 cycle/row; data is DMA'd straight into
float32r tiles (PE rounds internally) or produced by compute ops with float32r
output dtype.

Perf notes (evolved over NTFF traces; 355us -> ~291us/core, rel err 1.7e-3):
 - attention uses 1-bank [128,512] score psum tiles (bufs=4 pipeline), one exp
   per 512-half (starts as soon as its half is ready), one dacc add per key
   block covering the full 1024-wide pair.
 - each pair's normalize/store tail is deferred into the NEXT pair's stream
   (emitted after its step 2), removing all pair-boundary PE stalls; final
   trace shows 2us total PE idle, max gap 225ns, K=8/8 end to end.
 - HAM warmup: ~28 dependency-free matmuls on zeroed tiles bridge the initial
   DMA window so the PE clock gate arms (K=8/8) before real work; low-duty or
   gappy PE phases re-throttle to 1.2 GHz for 3.4us-quantized windows (earlier
   versions lost 38-72us to this).
 - projections are dissolved into pair 0's key-block stream: QT/KT/V live in
   per-1024-column piece tiles and each piece is emitted just-in-time a few
   steps before the attention blocks that consume it, so the PE always has
   dense 512-free matmul work and never waits on a DMA/projection tail.
 - weights/biases are pre-swizzled on host so every input DMA is a plain 2D
   contiguous transfer; input DMAs are ordered by first consumption.
 - per-co U accumulators + per-512 normalize/store shrink pair tails.
 - reciprocal_approx_fast (18 bits) for the softmax denominators.
 - measured on HW: PE streams the 1024 attention matmuls back-to-back at
   ~227ns each (512 free columns; bf16 measures 216ns, not worth the
   precision); kernel span is within ~10% of that floor plus the fixed
   ~8us preamble and ~10us drain epilogue.
"""

import numpy as np

import concourse.bacc as bacc
import concourse.mybir as mybir
import concourse.tile as tile
from concourse.bass_utils import run_bass_kernel_spmd

F32 = mybir.dt.float32
F32R = mybir.dt.float32r
AF = mybir.ActivationFunctionType

B, C, H, W = 4, 256, 64, 64
N = H * W            # 4096 tokens
CK = C // 128        # 2 channel chunks of 128
NT = 512             # attention n-tile (query block, free dim)
NP = 1024            # n-tile pair width
N_PAIR = N // NP     # 4
MB = 128             # key/value block (partition block)
N_MB = N // MB       # 32
MB_PER_PIECE = NP // MB   # 8 key blocks per kt/v piece
SKEW = 3             # software-pipeline skew between S and U matmuls

_NC_CACHE = None
LAST_RESULT = None   # BassKernelResults of the most recent kernel() call


def _build():
    nc = bacc.Bacc("TRN2", target_bir_lowering=False, debug=False)

    xa = nc.dram_tensor("xa", [C, N], F32, kind="ExternalInput").ap()
    xb = nc.dram_tensor("xb", [C, N], F32, kind="ExternalInput").ap()
    # host pre-swizzled: [128, CK*C] with (ki, co*128+j) element order
    wq = nc.dram_tensor("wq_l", [128, CK * C], F32, kind="ExternalInput").ap()
    wk = nc.dram_tensor("wk_l", [128, CK * C], F32, kind="ExternalInput").ap()
    wv = nc.dram_tensor("wv_l", [128, CK * C], F32, kind="ExternalInput").ap()
    bq = nc.dram_tensor("bq_l", [128, CK], F32, kind="ExternalInput").ap()
    bk = nc.dram_tensor("bk_l", [128, CK], F32, kind="ExternalInput").ap()
    bv = nc.dram_tensor("bv_l", [1, C], F32, kind="ExternalInput").ap()
    out = nc.dram_tensor("o", [C, N], F32, kind="ExternalOutput").ap()

    with tile.TileContext(nc) as tc:
        with tc.tile_pool(name="persist", bufs=1) as pp, \
             tc.tile_pool(name="xbs", bufs=4) as xbp, \
             tc.tile_pool(name="os", bufs=3) as op_, \
             tc.tile_pool(name="s_ps", bufs=4, space="PSUM") as sp, \
             tc.tile_pool(name="u_ps", bufs=1, space="PSUM") as up, \
             tc.tile_pool(name="e_sb", bufs=4) as ep, \
             tc.tile_pool(name="acc", bufs=2) as ap_:
            # ---- HAM warmup (emitted first, zero data deps) -----------
            warm_w = pp.tile([128, 128], F32R, tag="warm_w")
            warm_src = pp.tile([128, NT], F32R, tag="warm_src")
            nc.vector.memset(warm_w[:].bitcast(F32), 0.0)
            nc.vector.memset(warm_src[:].bitcast(F32), 0.0)
            warm_ps = sp.tile([128, NT], F32, tag="s")
            N_WARM = 26
            for it in range(N_WARM):
                nc.tensor.matmul(warm_ps[:], warm_w[:], warm_src[:],
                                 start=(it == 0), stop=(it == N_WARM - 1))

            # ---- parameters & inputs, in consumption order ------------
            w_r = {}
            for name in ("wq", "wk", "wv"):
                w_r[name] = pp.tile([128, CK, C], F32R, name=f"{name}_r",
                                    tag=f"{name}_r")
            bq_sb = pp.tile([128, CK], F32, tag="bq_sb")
            bk_sb = pp.tile([128, CK], F32, tag="bk_sb")
            bv_r = pp.tile([1, C], F32R, tag="bv_r")
            xa_pieces = {}
            xa_q = {}
            for ki in range(CK):
                for h in range(2):
                    xa_q[(ki, h)] = pp.tile(
                        [128, NT], F32R, name=f"xaq_{ki}_{h}", tag=f"xaq_{ki}_{h}")
            for pc in range(1, 4):
                for ki in range(CK):
                    xa_pieces[(ki, pc)] = pp.tile(
                        [128, NP], F32R, name=f"xa_{ki}_{pc}", tag=f"xa_{ki}_{pc}")

            def xa_rhs(ki, nt):
                # 512-wide rhs slice of xa for QT tile nt
                if nt < 2:
                    return xa_q[(ki, nt)][:]
                piece = xa_pieces[(ki, nt // 2)]
                return piece[:, (nt % 2) * NT:((nt % 2) + 1) * NT]

            def xa_mb(ki, mb):
                # 128-wide lhsT slice of xa for V block mb
                if mb < MB_PER_PIECE:
                    t = xa_q[(ki, mb // 4)]
                    return t[:, (mb % 4) * 128:((mb % 4) + 1) * 128]
                piece = xa_pieces[(ki, mb // MB_PER_PIECE)]
                off = (mb % MB_PER_PIECE) * 128
                return piece[:, off:off + 128]
            xb_pieces = {}
            for pc in range(4):
                for ki in range(CK):
                    xb_pieces[(ki, pc)] = xbp.tile(
                        [128, NP], F32R, name=f"xb_{ki}_{pc}", tag="xb")

            def dma_x(pieces, src, ki, pc):
                nc.sync.dma_start(
                    pieces[(ki, pc)][:],
                    src[ki * 128:(ki + 1) * 128,
                        pc * NP:(pc + 1) * NP].bitcast(F32R))

            nc.sync.dma_start(w_r["wq"][:].rearrange("p k m -> p (k m)"),
                              wq.bitcast(F32R))
            nc.sync.dma_start(bq_sb[:], bq[:])
            for h in range(2):
                for ki in range(CK):
                    nc.sync.dma_start(
                        xa_q[(ki, h)][:],
                        xa[ki * 128:(ki + 1) * 128,
                           h * NT:(h + 1) * NT].bitcast(F32R))
            nc.sync.dma_start(w_r["wv"][:].rearrange("p k m -> p (k m)"),
                              wv.bitcast(F32R))
            nc.sync.dma_start(bv_r[:], bv.bitcast(F32R))
            nc.sync.dma_start(w_r["wk"][:].rearrange("p k m -> p (k m)"),
                              wk.bitcast(F32R))
            nc.sync.dma_start(bk_sb[:], bk[:])
            dma_x(xb_pieces, xb, 0, 0)
            dma_x(xb_pieces, xb, 1, 0)
            for pc in range(1, 4):
                dma_x(xa_pieces, xa, 0, pc)
                dma_x(xa_pieces, xa, 1, pc)
                dma_x(xb_pieces, xb, 0, pc)
                dma_x(xb_pieces, xb, 1, pc)

            ones_f = pp.tile([128, 128], F32, tag="ones_f")
            nc.vector.memset(ones_f[:], 1.0)
            ones_r = pp.tile([128, 128], F32R, tag="ones_r")
            nc.vector.tensor_copy(ones_r[:], ones_f[:])

            # bv replicated to all partitions once (K=1 ones matmul), so the
            # per-block V bias is a DVE add instead of an extra tiny matmul
            bv_ps = sp.tile([128, NT], F32, tag="s")
            nc.tensor.matmul(bv_ps[:, 0:C], ones_r[0:1, :], bv_r[:],
                             start=True, stop=True)
            bv_rep = pp.tile([128, CK, 128], F32, tag="bv_rep")
            nc.vector.tensor_copy(
                bv_rep[:], bv_ps[:, 0:C].rearrange("p (c j) -> p c j", c=CK))

            # per-pair piece tiles for QT / KT / V
            qt_p = [pp.tile([128, CK, NP], F32R, name=f"qt_{p}", tag=f"qt_{p}")
                    for p in range(N_PAIR)]
            kt_p = [pp.tile([128, CK, NP], F32R, name=f"kt_{p}", tag=f"kt_{p}")
                    for p in range(N_PAIR)]
            v_p = [pp.tile([128, CK, NP], F32R, name=f"v_{p}", tag=f"v_{p}")
                   for p in range(N_PAIR)]

            # ---- projection emitters ---------------------------------
            def emit_qtkt(dst_p, w_t, b_sb, pieces, co, nt):
                ps = sp.tile([128, NT], F32, tag="s")
                half = ps[:]
                for ki in range(CK):
                    if pieces is xa_pieces:
                        rhs = xa_rhs(ki, nt)
                    else:
                        piece = pieces[(ki, nt // 2)]
                        rhs = piece[:, (nt % 2) * NT:((nt % 2) + 1) * NT]
                    nc.tensor.matmul(
                        half, w_t[:, ki, co * 128:(co + 1) * 128],
                        rhs, start=(ki == 0), stop=(ki == CK - 1))
                nc.vector.tensor_scalar_add(
                    dst_p[nt // 2][:, co, (nt % 2) * NT:((nt % 2) + 1) * NT],
                    half, b_sb[:, co:co + 1])

            def emit_v(mb):
                ps = sp.tile([128, NT], F32, tag="s")
                half = ps[:, 0:C]
                for ki in range(CK):
                    nc.tensor.matmul(
                        half, xa_mb(ki, mb), w_r["wv"][:, ki, :],
                        start=(ki == 0), stop=(ki == CK - 1))
                off = (mb % MB_PER_PIECE) * 128
                nc.vector.tensor_add(
                    v_p[mb // MB_PER_PIECE][:, :, off:off + 128],
                    half.rearrange("p (c j) -> p c j", c=CK), bv_rep[:])

            # ---- attention emitters ----------------------------------
            pair_state = {}

            def attn_begin(pc):
                pair_state[pc] = {
                    "u": [up.tile([128, NP], F32, name=f"u_{pc}_{co}",
                                  tag=f"u{co}") for co in range(CK)],
                    "dacc": ap_.tile([128, NP], F32R, name=f"dacc_{pc}",
                                     tag="dacc"),
                    "e": {},
                }

            def attn_step(pc, step):
                st = pair_state[pc]
                if step < N_MB:
                    mb = step
                    s_h = [sp.tile([128, NT], F32, name=f"s_h{ho}", tag="s")
                           for ho in range(2)]
                    ktp = kt_p[mb // MB_PER_PIECE]
                    off = (mb % MB_PER_PIECE) * 128
                    for co in range(CK):
                        for ho in range(2):
                            nc.tensor.matmul(
                                s_h[ho][:],
                                ktp[:, co, off:off + 128],
                                qt_p[pc][:, co, ho * NT:(ho + 1) * NT],
                                start=(co == 0), stop=(co == CK - 1))
                    e_r = ep.tile([128, NP], F32R, tag="e")
                    for ho in range(2):
                        nc.scalar.activation(
                            e_r[:, ho * NT:(ho + 1) * NT], s_h[ho][:], AF.Exp)
                    st["e"][mb] = e_r
                if step >= SKEW:
                    mb = step - SKEW
                    e_r = st["e"].pop(mb)
                    vp = v_p[mb // MB_PER_PIECE]
                    off = (mb % MB_PER_PIECE) * 128
                    for co in range(CK):
                        for ho in range(2):
                            nc.tensor.matmul(
                                st["u"][co][:, ho * NT:(ho + 1) * NT],
                                vp[:, co, off:off + 128],
                                e_r[:, ho * NT:(ho + 1) * NT],
                                start=(mb == 0), stop=(mb == N_MB - 1))
                    if mb == 0:
                        nc.vector.tensor_copy(st["dacc"][:], e_r[:])
                    else:
                        nc.vector.tensor_add(st["dacc"][:], st["dacc"][:], e_r[:])

            def attn_end(pc):
                # per-512-half normalize + store so the output DMA of one half
                # overlaps the multiply of the next (shrinks the serial tail)
                st = pair_state.pop(pc)
                dinv = ap_.tile([128, NP], F32, name=f"dinv_{pc}", tag="dinv")
                for ho in range(2):
                    d_ps = sp.tile([128, NT], F32, name=f"d_{ho}", tag="s")
                    nc.tensor.matmul(d_ps[:], ones_r[:],
                                     st["dacc"][:, ho * NT:(ho + 1) * NT],
                                     start=True, stop=True)
                    nc.vector.reciprocal_approx_fast(
                        dinv[:, ho * NT:(ho + 1) * NT], d_ps[:])
                for co in range(CK):
                    for ho in range(2):
                        o_sb = op_.tile([128, NT], F32, tag="o_sb")
                        nc.vector.tensor_mul(
                            o_sb[:], st["u"][co][:, ho * NT:(ho + 1) * NT],
                            dinv[:, ho * NT:(ho + 1) * NT])
                        nc.sync.dma_start(
                            out[co * 128:(co + 1) * 128,
                                pc * NP + ho * NT:pc * NP + (ho + 1) * NT],
                            o_sb[:])

            # ---- emission schedule -----------------------------------
            # Most projection work is distributed just-in-time through pair
            # 0's key-block stream: the PE then always has dense 512-free
            # matmul work and the HAM clock gate never re-throttles (the
            # standalone projection phase ran at 45% duty -> K=4/8 for ~48us
            # in v5). Only the piece-0 prerequisites run up front.
            def emit_qt(co, nt):
                emit_qtkt(qt_p, w_r["wq"], bq_sb, xa_pieces, co, nt)

            def emit_kt(co, nt):
                emit_qtkt(kt_p, w_r["wk"], bk_sb, xb_pieces, co, nt)

            # prerequisites for pair 0, step 0..5
            for co in range(CK):
                for nto in range(2):
                    emit_qt(co, nto)
            for mb in range(6):
                emit_v(mb)
            for co in range(CK):
                for nto in range(2):
                    emit_kt(co, nto)

            # just-in-time jobs sprinkled through pair 0's steps
            extra = {}

            def add_extra(step, fn):
                extra.setdefault(step, []).append(fn)

            for p in range(1, 4):           # kt piece p before S reads it
                for i, (co, nto) in enumerate(
                        ((0, 0), (0, 1), (1, 0), (1, 1))):
                    add_extra(p * MB_PER_PIECE - 6 + i,
                              lambda co=co, nt=2 * p + nto: emit_kt(co, nt))
            for mb in range(6, N_MB):       # v block 4 steps ahead of its U
                add_extra(mb - 4, lambda mb=mb: emit_v(mb))
            for p in range(1, 4):           # qt pieces 1-3 anywhere in pair 0
                for i, (co, nto) in enumerate(
                        ((0, 0), (0, 1), (1, 0), (1, 1))):
                    add_extra(2 + p * 6 + i,
                              lambda co=co, nt=2 * p + nto: emit_qt(co, nt))

            attn_begin(0)
            for step in range(N_MB + SKEW):
                attn_step(0, step)
                for fn in extra.pop(step, ()):
                    fn()
            for pc in range(1, N_PAIR):
                attn_begin(pc)
                for step in range(N_MB + SKEW):
                    attn_step(pc, step)
                    if step == 2:
                        attn_end(pc - 1)
            attn_end(N_PAIR - 1)
    nc.compile()
    return nc


def _get_nc():
    global _NC_CACHE
    if _NC_CACHE is None:
        _NC_CACHE = _build()
    return _NC_CACHE


def _w_layout(w):
    # lhsT chunks: w_l[p, ki*C + m] = W.T[ki*128 + p, m]
    wt = np.ascontiguousarray(np.asarray(w, np.float32).T)      # [C_in, C_out]
    return np.ascontiguousarray(
        wt.reshape(CK, 128, C).transpose(1, 0, 2).reshape(128, CK * C))


def kernel(x1, x2, Wq, bq, Wk, bk, Wv, bv):
    global LAST_RESULT
    x1 = np.asarray(x1, dtype=np.float32)
    x2 = np.asarray(x2, dtype=np.float32)
    shared = {
        "wq_l": _w_layout(Wq),
        "wk_l": _w_layout(Wk),
        "wv_l": _w_layout(Wv),
        "bq_l": np.ascontiguousarray(
            np.asarray(bq, np.float32).reshape(CK, 128).T),
        "bk_l": np.ascontiguousarray(
            np.asarray(bk, np.float32).reshape(CK, 128).T),
        "bv_l": np.asarray(bv, np.float32).reshape(1, C),
    }
    in_maps = []
    for core in range(8):
        b, s = core % B, core // B
        xs, xo = (x1, x2) if s == 0 else (x2, x1)
        in_maps.append({
            "xa": np.ascontiguousarray(xs[b].reshape(C, N)),
            "xb": np.ascontiguousarray(xo[b].reshape(C, N)),
            **shared,
        })
    nc = _get_nc()
    res = run_bass_kernel_spmd(nc, in_maps, list(range(8)))
    LAST_RESULT = res
    x1_out = np.stack([res.results[b]["o"].reshape(C, H, W) for b in range(B)])
    x2_out = np.stack([res.results[B + b]["o"].reshape(C, H, W) for b in range(B)])
    return (x1_out, x2_out)


# revision 28
# speedup vs baseline: 1.0023x; 1.0023x over previous
"""Cross-temporal attention Trainium2 (Bass/Tile) kernel.

Problem: two streams x1, x2 of shape [B=4, C=256, H=64, W=64]; tokens are the
H*W=4096 spatial positions. Per batch b and stream s:
    q_s = t_s @ Wq.T + bq ; k_s = t_s @ Wk.T + bk ; v_s = t_s @ Wv.T + bv
    out_s = softmax(q_s @ k_{3-s}.T) @ v_s            (no 1/sqrt(d) scaling)

Sharding: 8 NeuronCores, one (batch, stream) unit per core (4 batches x 2
streams). Fully SPMD — the same program runs on every core, only the input
bindings differ. No collectives.

Per-core layout trick: x[b] is already [C, N] channel-major, which is exactly
the transposed token matrix. All intermediates stay transposed:
    QT = Wq @ X + bq   [C, N]      (PE: lhsT = Wq^T chunks, rhs = X chunks)
    KT = Wk @ Xo + bk  [C, N]
    V  = X^T @ Wv^T + bv  [N, C]   (PE: lhsT = X chunks, rhs = Wv^T)
    ST = KT^T-block @ QT = scores^T  [m, n] blocks   (softmax over m = partitions)
    E  = exp(ST)   (no max subtraction: |logits| < ~40 << 88, fp32-safe)
    U  = accum_m V^T-block @ E  -> [C, n] unnormalized out^T
    D  = column sums of E (ones-matmul replicates to all partitions)
    OT = U / D     [C, N] == x_out[b] flattened. No transposes anywhere.

All matmuls run in float32r (TF32) at 1 cycle/row; data is DMA'd straight into
float32r tiles (PE rounds internally) or produced by compute ops with float32r
output dtype.

Perf notes (evolved over NTFF traces; 355us -> ~292us/core, rel err 1.7e-3):
 - attention uses 1-bank [128,512] score psum tiles (bufs=4 pipeline), one exp
   per 512-half (starts as soon as its half is ready), one dacc add per key
   block covering the full 1024-wide pair.
 - each pair's normalize/store tail is deferred into the NEXT pair's stream
   (emitted after its step 2), removing all pair-boundary PE stalls; final
   trace shows 2us total PE idle, max gap 225ns, K=8/8 end to end.
 - HAM warmup: ~26 dependency-free matmuls on zeroed tiles bridge the initial
   DMA window so the PE clock gate arms (K=8/8) before real work; low-duty or
   gappy PE phases re-throttle to 1.2 GHz for 3.4us-quantized windows (earlier
   versions lost 38-72us to this). The first xa pieces are 512-wide so the
   first projection group unblocks as early as possible.
 - projections are dissolved into pair 0's key-block stream: QT/KT/V live in
   per-1024-column piece tiles and each piece is emitted just-in-time a few
   steps before the attention blocks that consume it, so the PE always has
   dense 512-free matmul work and never waits on a DMA/projection tail.
 - weights/biases are pre-swizzled on host so every input DMA is a plain 2D
   contiguous transfer; input DMAs are ordered by first consumption.
 - per-co U accumulators + per-512 normalize/store shrink pair tails.
 - reciprocal_approx_fast (18 bits) for the softmax denominators.
 - measured on HW: PE streams the 1024 attention matmuls back-to-back at
   ~227ns each (512 free columns; bf16 measures 216ns, not worth the
   precision); kernel span is within ~10% of that floor plus the fixed
   ~8us preamble and ~10us drain epilogue.
"""

import numpy as np

import concourse.bacc as bacc
import concourse.mybir as mybir
import concourse.tile as tile
from concourse.bass_utils import run_bass_kernel_spmd

F32 = mybir.dt.float32
F32R = mybir.dt.float32r
AF = mybir.ActivationFunctionType

B, C, H, W = 4, 256, 64, 64
N = H * W            # 4096 tokens
CK = C // 128        # 2 channel chunks of 128
NT = 512             # attention n-tile (query block, free dim)
NP = 1024            # n-tile pair width
N_PAIR = N // NP     # 4
MB = 128             # key/value block (partition block)
N_MB = N // MB       # 32
MB_PER_PIECE = NP // MB   # 8 key blocks per kt/v piece
SKEW = 3             # software-pipeline skew between S and U matmuls

_NC_CACHE = None
LAST_RESULT = None   # BassKernelResults of the most recent kernel() call


def _build():
    nc = bacc.Bacc("TRN2", target_bir_lowering=False, debug=False)

    xa = nc.dram_tensor("xa", [C, N], F32, kind="ExternalInput").ap()
    xb = nc.dram_tensor("xb", [C, N], F32, kind="ExternalInput").ap()
    # host pre-swizzled: [128, CK*C] with (ki, co*128+j) element order
    wq = nc.dram_tensor("wq_l", [128, CK * C], F32, kind="ExternalInput").ap()
    wk = nc.dram_tensor("wk_l", [128, CK * C], F32, kind="ExternalInput").ap()
    wv = nc.dram_tensor("wv_l", [128, CK * C], F32, kind="ExternalInput").ap()
    bq = nc.dram_tensor("bq_l", [128, CK], F32, kind="ExternalInput").ap()
    bk = nc.dram_tensor("bk_l", [128, CK], F32, kind="ExternalInput").ap()
    bv = nc.dram_tensor("bv_l", [1, C], F32, kind="ExternalInput").ap()
    out = nc.dram_tensor("o", [C, N], F32, kind="ExternalOutput").ap()

    with tile.TileContext(nc) as tc:
        with tc.tile_pool(name="persist", bufs=1) as pp, \
             tc.tile_pool(name="xbs", bufs=4) as xbp, \
             tc.tile_pool(name="os", bufs=3) as op_, \
             tc.tile_pool(name="s_ps", bufs=4, space="PSUM") as sp, \
             tc.tile_pool(name="u_ps", bufs=1, space="PSUM") as up, \
             tc.tile_pool(name="e_sb", bufs=4) as ep, \
             tc.tile_pool(name="acc", bufs=2) as ap_:
            # ---- HAM warmup (emitted first, zero data deps) -----------
            warm_w = pp.tile([128, 128], F32R, tag="warm_w")
            warm_src = pp.tile([128, NT], F32R, tag="warm_src")
            nc.vector.memset(warm_w[:].bitcast(F32), 0.0)
            nc.vector.memset(warm_src[:].bitcast(F32), 0.0)
            warm_ps = sp.tile([128, NT], F32, tag="s")
            N_WARM = 26
            for it in range(N_WARM):
                nc.tensor.matmul(warm_ps[:], warm_w[:], warm_src[:],
                                 start=(it == 0), stop=(it == N_WARM - 1))

            # ---- parameters & inputs, in consumption order ------------
            w_r = {}
            for name in ("wq", "wk", "wv"):
                w_r[name] = pp.tile([128, CK, C], F32R, name=f"{name}_r",
                                    tag=f"{name}_r")
            bq_sb = pp.tile([128, CK], F32, tag="bq_sb")
            bk_sb = pp.tile([128, CK], F32, tag="bk_sb")
            bv_r = pp.tile([1, C], F32R, tag="bv_r")
            xa_pieces = {}
            xa_q = {}
            for ki in range(CK):
                for h in range(2):
                    xa_q[(ki, h)] = pp.tile(
                        [128, NT], F32R, name=f"xaq_{ki}_{h}", tag=f"xaq_{ki}_{h}")
            for pc in range(1, 4):
                for ki in range(CK):
                    xa_pieces[(ki, pc)] = pp.tile(
                        [128, NP], F32R, name=f"xa_{ki}_{pc}", tag=f"xa_{ki}_{pc}")

            def xa_rhs(ki, nt):
                # 512-wide rhs slice of xa for QT tile nt
                if nt < 2:
                    return xa_q[(ki, nt)][:]
                piece = xa_pieces[(ki, nt // 2)]
                return piece[:, (nt % 2) * NT:((nt % 2) + 1) * NT]

            def xa_mb(ki, mb):
                # 128-wide lhsT slice of xa for V block mb
                if mb < MB_PER_PIECE:
                    t = xa_q[(ki, mb // 4)]
                    return t[:, (mb % 4) * 128:((mb % 4) + 1) * 128]
                piece = xa_pieces[(ki, mb // MB_PER_PIECE)]
                off = (mb % MB_PER_PIECE) * 128
                return piece[:, off:off + 128]
            xb_pieces = {}
            for pc in range(4):
                for ki in range(CK):
                    xb_pieces[(ki, pc)] = xbp.tile(
                        [128, NP], F32R, name=f"xb_{ki}_{pc}", tag="xb")

            def dma_x(pieces, src, ki, pc):
                nc.sync.dma_start(
                    pieces[(ki, pc)][:],
                    src[ki * 128:(ki + 1) * 128,
                        pc * NP:(pc + 1) * NP].bitcast(F32R))

            nc.sync.dma_start(w_r["wq"][:].rearrange("p k m -> p (k m)"),
                              wq.bitcast(F32R))
            nc.sync.dma_start(bq_sb[:], bq[:])
            for h in range(2):
                for ki in range(CK):
                    nc.sync.dma_start(
                        xa_q[(ki, h)][:],
                        xa[ki * 128:(ki + 1) * 128,
                           h * NT:(h + 1) * NT].bitcast(F32R))
            nc.sync.dma_start(w_r["wv"][:].rearrange("p k m -> p (k m)"),
                              wv.bitcast(F32R))
            nc.sync.dma_start(bv_r[:], bv.bitcast(F32R))
            nc.sync.dma_start(w_r["wk"][:].rearrange("p k m -> p (k m)"),
                              wk.bitcast(F32R))
            nc.sync.dma_start(bk_sb[:], bk[:])
            dma_x(xb_pieces, xb, 0, 0)
            dma_x(xb_pieces, xb, 1, 0)
            for pc in range(1, 4):
                dma_x(xa_pieces, xa, 0, pc)
                dma_x(xa_pieces, xa, 1, pc)
                dma_x(xb_pieces, xb, 0, pc)
                dma_x(xb_pieces, xb, 1, pc)

            ones_f = pp.tile([128, 128], F32, tag="ones_f")
            nc.vector.memset(ones_f[:], 1.0)
            ones_r = pp.tile([128, 128], F32R, tag="ones_r")
            nc.vector.tensor_copy(ones_r[:], ones_f[:])

            # bv replicated to all partitions once (K=1 ones matmul), so the
            # per-block V bias is a DVE add instead of an extra tiny matmul
            bv_ps = sp.tile([128, NT], F32, tag="s")
            nc.tensor.matmul(bv_ps[:, 0:C], ones_r[0:1, :], bv_r[:],
                             start=True, stop=True)
            bv_rep = pp.tile([128, CK, 128], F32, tag="bv_rep")
            nc.vector.tensor_copy(
                bv_rep[:], bv_ps[:, 0:C].rearrange("p (c j) -> p c j", c=CK))

            # per-pair piece tiles for QT / KT / V
            qt_p = [pp.tile([128, CK, NP], F32R, name=f"qt_{p}", tag=f"qt_{p}")
                    for p in range(N_PAIR)]
            kt_p = [pp.tile([128, CK, NP], F32R, name=f"kt_{p}", tag=f"kt_{p}")
                    for p in range(N_PAIR)]
            v_p = [pp.tile([128, CK, NP], F32R, name=f"v_{p}", tag=f"v_{p}")
                   for p in range(N_PAIR)]

            # ---- projection emitters ---------------------------------
            def emit_qtkt(dst_p, w_t, b_sb, pieces, co, nt):
                ps = sp.tile([128, NT], F32, tag="s")
                half = ps[:]
                for ki in range(CK):
                    if pieces is xa_pieces:
                        rhs = xa_rhs(ki, nt)
                    else:
                        piece = pieces[(ki, nt // 2)]
                        rhs = piece[:, (nt % 2) * NT:((nt % 2) + 1) * NT]
                    nc.tensor.matmul(
                        half, w_t[:, ki, co * 128:(co + 1) * 128],
                        rhs, start=(ki == 0), stop=(ki == CK - 1))
                nc.vector.tensor_scalar_add(
                    dst_p[nt // 2][:, co, (nt % 2) * NT:((nt % 2) + 1) * NT],
                    half, b_sb[:, co:co + 1])

            def emit_v(mb):
                ps = sp.tile([128, NT], F32, tag="s")
                half = ps[:, 0:C]
                for ki in range(CK):
                    nc.tensor.matmul(
                        half, xa_mb(ki, mb), w_r["wv"][:, ki, :],
                        start=(ki == 0), stop=(ki == CK - 1))
                off = (mb % MB_PER_PIECE) * 128
                nc.vector.tensor_add(
                    v_p[mb // MB_PER_PIECE][:, :, off:off + 128],
                    half.rearrange("p (c j) -> p c j", c=CK), bv_rep[:])

            # ---- attention emitters ----------------------------------
            pair_state = {}

            def attn_begin(pc):
                pair_state[pc] = {
                    "u": [up.tile([128, NP], F32, name=f"u_{pc}_{co}",
                                  tag=f"u{co}") for co in range(CK)],
                    "dacc": ap_.tile([128, NP], F32R, name=f"dacc_{pc}",
                                     tag="dacc"),
                    "e": {},
                }

            def attn_step(pc, step):
                st = pair_state[pc]
                if step < N_MB:
                    mb = step
                    s_h = [sp.tile([128, NT], F32, name=f"s_h{ho}", tag="s")
                           for ho in range(2)]
                    ktp = kt_p[mb // MB_PER_PIECE]
                    off = (mb % MB_PER_PIECE) * 128
                    for co in range(CK):
                        for ho in range(2):
                            nc.tensor.matmul(
                                s_h[ho][:],
                                ktp[:, co, off:off + 128],
                                qt_p[pc][:, co, ho * NT:(ho + 1) * NT],
                                start=(co == 0), stop=(co == CK - 1))
                    e_r = ep.tile([128, NP], F32R, tag="e")
                    for ho in range(2):
                        nc.scalar.activation(
                            e_r[:, ho * NT:(ho + 1) * NT], s_h[ho][:], AF.Exp)
                    st["e"][mb] = e_r
                if step >= SKEW:
                    mb = step - SKEW
                    e_r = st["e"].pop(mb)
                    vp = v_p[mb // MB_PER_PIECE]
                    off = (mb % MB_PER_PIECE) * 128
                    for co in range(CK):
                        for ho in range(2):
                            nc.tensor.matmul(
                                st["u"][co][:, ho * NT:(ho + 1) * NT],
                                vp[:, co, off:off + 128],
                                e_r[:, ho * NT:(ho + 1) * NT],
                                start=(mb == 0), stop=(mb == N_MB - 1))
                    if mb == 0:
                        nc.vector.tensor_copy(st["dacc"][:], e_r[:])
                    else:
                        nc.vector.tensor_add(st["dacc"][:], st["dacc"][:], e_r[:])

            def attn_end(pc):
                # per-512-half normalize + store so the output DMA of one half
                # overlaps the multiply of the next (shrinks the serial tail)
                st = pair_state.pop(pc)
                dinv = ap_.tile([128, NP], F32, name=f"dinv_{pc}", tag="dinv")
                for ho in range(2):
                    d_ps = sp.tile([128, NT], F32, name=f"d_{ho}", tag="s")
                    nc.tensor.matmul(d_ps[:], ones_r[:],
                                     st["dacc"][:, ho * NT:(ho + 1) * NT],
                                     start=True, stop=True)
                    nc.vector.reciprocal_approx_fast(
                        dinv[:, ho * NT:(ho + 1) * NT], d_ps[:])
                for co in range(CK):
                    for ho in range(2):
                        o_sb = op_.tile([128, NT], F32, tag="o_sb")
                        nc.vector.tensor_mul(
                            o_sb[:], st["u"][co][:, ho * NT:(ho + 1) * NT],
                            dinv[:, ho * NT:(ho + 1) * NT])
                        nc.sync.dma_start(
                            out[co * 128:(co + 1) * 128,
                                pc * NP + ho * NT:pc * NP + (ho + 1) * NT],
                            o_sb[:])

            # ---- emission schedule -----------------------------------
            # Most projection work is distributed just-in-time through pair
            # 0's key-block stream: the PE then always has dense 512-free
            # matmul work and the HAM clock gate never re-throttles (the
            # standalone projection phase ran at 45% duty -> K=4/8 for ~48us
            # in v5). Only the piece-0 prerequisites run up front.
            def emit_qt(co, nt):
                emit_qtkt(qt_p, w_r["wq"], bq_sb, xa_pieces, co, nt)

            def emit_kt(co, nt):
                emit_qtkt(kt_p, w_r["wk"], bk_sb, xb_pieces, co, nt)

            # prerequisites for pair 0, step 0..5
            for co in range(CK):
                for nto in range(2):
                    emit_qt(co, nto)
            for mb in range(8):
                emit_v(mb)
            for co in range(CK):
                for nto in range(2):
                    emit_kt(co, nto)

            # just-in-time jobs sprinkled through pair 0's steps
            extra = {}

            def add_extra(step, fn):
                extra.setdefault(step, []).append(fn)

            for p in range(1, 4):           # kt piece p before S reads it
                for i, (co, nto) in enumerate(
                        ((0, 0), (0, 1), (1, 0), (1, 1))):
                    add_extra(p * MB_PER_PIECE - 6 + i,
                              lambda co=co, nt=2 * p + nto: emit_kt(co, nt))
            for mb in range(8, N_MB):       # v block 4 steps ahead of its U
                add_extra(mb - 4, lambda mb=mb: emit_v(mb))
            for p in range(1, 4):           # qt pieces 1-3 anywhere in pair 0
                for i, (co, nto) in enumerate(
                        ((0, 0), (0, 1), (1, 0), (1, 1))):
                    add_extra(2 + p * 6 + i,
                              lambda co=co, nt=2 * p + nto: emit_qt(co, nt))

            attn_begin(0)
            for step in range(N_MB + SKEW):
                attn_step(0, step)
                for fn in extra.pop(step, ()):
                    fn()
            for pc in range(1, N_PAIR):
                attn_begin(pc)
                for step in range(N_MB + SKEW):
                    attn_step(pc, step)
                    if step == 2:
                        attn_end(pc - 1)
            attn_end(N_PAIR - 1)
    nc.compile()
    return nc


def _get_nc():
    global _NC_CACHE
    if _NC_CACHE is None:
        _NC_CACHE = _build()
    return _NC_CACHE


def _w_layout(w):
    # lhsT chunks: w_l[p, ki*C + m] = W.T[ki*128 + p, m]
    wt = np.ascontiguousarray(np.asarray(w, np.float32).T)      # [C_in, C_out]
    return np.ascontiguousarray(
        wt.reshape(CK, 128, C).transpose(1, 0, 2).reshape(128, CK * C))


def kernel(x1, x2, Wq, bq, Wk, bk, Wv, bv):
    global LAST_RESULT
    x1 = np.asarray(x1, dtype=np.float32)
    x2 = np.asarray(x2, dtype=np.float32)
    shared = {
        "wq_l": _w_layout(Wq),
        "wk_l": _w_layout(Wk),
        "wv_l": _w_layout(Wv),
        "bq_l": np.ascontiguousarray(
            np.asarray(bq, np.float32).reshape(CK, 128).T),
        "bk_l": np.ascontiguousarray(
            np.asarray(bk, np.float32).reshape(CK, 128).T),
        "bv_l": np.asarray(bv, np.float32).reshape(1, C),
    }
    in_maps = []
    for core in range(8):
        b, s = core % B, core // B
        xs, xo = (x1, x2) if s == 0 else (x2, x1)
        in_maps.append({
            "xa": np.ascontiguousarray(xs[b].reshape(C, N)),
            "xb": np.ascontiguousarray(xo[b].reshape(C, N)),
            **shared,
        })
    nc = _get_nc()
    res = run_bass_kernel_spmd(nc, in_maps, list(range(8)))
    LAST_RESULT = res
    x1_out = np.stack([res.results[b]["o"].reshape(C, H, W) for b in range(B)])
    x2_out = np.stack([res.results[B + b]["o"].reshape(C, H, W) for b in range(B)])
    return (x1_out, x2_out)


# revision 30
# speedup vs baseline: 1.0096x; 1.0073x over previous
"""Cross-temporal attention Trainium2 (Bass/Tile) kernel.

Problem: two streams x1, x2 of shape [B=4, C=256, H=64, W=64]; tokens are the
H*W=4096 spatial positions. Per batch b and stream s:
    q_s = t_s @ Wq.T + bq ; k_s = t_s @ Wk.T + bk ; v_s = t_s @ Wv.T + bv
    out_s = softmax(q_s @ k_{3-s}.T) @ v_s            (no 1/sqrt(d) scaling)

Sharding: 8 NeuronCores, one (batch, stream) unit per core (4 batches x 2
streams). Fully SPMD — the same program runs on every core, only the input
bindings differ. No collectives.

Per-core layout trick: x[b] is already [C, N] channel-major, which is exactly
the transposed token matrix. All intermediates stay transposed:
    QT = Wq @ X + bq   [C, N]      (PE: lhsT = Wq^T chunks, rhs = X chunks)
    KT = Wk @ Xo + bk  [C, N]
    V  = X^T @ Wv^T + bv  [N, C]   (PE: lhsT = X chunks, rhs = Wv^T)
    ST = KT^T-block @ QT = scores^T  [m, n] blocks   (softmax over m = partitions)
    E  = exp(ST)   (no max subtraction: |logits| < ~40 << 88, fp32-safe)
    U  = accum_m V^T-block @ E  -> [C, n] unnormalized out^T
    D  = column sums of E (ones-matmul replicates to all partitions)
    OT = U / D     [C, N] == x_out[b] flattened. No transposes anywhere.

All matmuls run in float32r (TF32) at 1 cycle/row; data is DMA'd straight into
float32r tiles (PE rounds internally) or produced by compute ops with float32r
output dtype.

Perf notes (evolved over NTFF traces; 355us -> ~292us/core, rel err 1.7e-3):
 - attention uses 1-bank [128,512] score psum tiles (bufs=4 pipeline), one exp
   per 512-half (starts as soon as its half is ready), one dacc add per key
   block covering the full 1024-wide pair.
 - each pair's normalize/store tail is deferred into the NEXT pair's stream
   (emitted after its step 2), removing all pair-boundary PE stalls; final
   trace shows 2us total PE idle, max gap 225ns, K=8/8 end to end.
 - HAM warmup: ~26 dependency-free matmuls on zeroed tiles bridge the initial
   DMA window so the PE clock gate arms (K=8/8) before real work; low-duty or
   gappy PE phases re-throttle to 1.2 GHz for 3.4us-quantized windows (earlier
   versions lost 38-72us to this). The first xa pieces are 512-wide so the
   first projection group unblocks as early as possible, and 8 V blocks run
   upfront so the PE has xa-gated work covering the xb piece-0 DMA wait.
 - projections are dissolved into pair 0's key-block stream: QT/KT/V live in
   per-1024-column piece tiles and each piece is emitted just-in-time a few
   steps before the attention blocks that consume it, so the PE always has
   dense 512-free matmul work and never waits on a DMA/projection tail.
 - weights/biases are pre-swizzled on host so every input DMA is a plain 2D
   contiguous transfer; input DMAs are ordered by first consumption.
 - per-co U accumulators + per-512 normalize/store shrink pair tails.
 - reciprocal_approx_fast (18 bits) for the softmax denominators.
 - measured on HW: PE streams the 1024 attention matmuls back-to-back at
   ~227ns each (512 free columns; bf16 measures 216ns, not worth the
   precision); kernel span is within ~10% of that floor plus the fixed
   ~8us preamble and ~10us drain epilogue.
"""

import numpy as np

import concourse.bacc as bacc
import concourse.mybir as mybir
import concourse.tile as tile
from concourse.bass_utils import run_bass_kernel_spmd

F32 = mybir.dt.float32
F32R = mybir.dt.float32r
AF = mybir.ActivationFunctionType

B, C, H, W = 4, 256, 64, 64
N = H * W            # 4096 tokens
CK = C // 128        # 2 channel chunks of 128
NT = 512             # attention n-tile (query block, free dim)
NP = 1024            # n-tile pair width
N_PAIR = N // NP     # 4
MB = 128             # key/value block (partition block)
N_MB = N // MB       # 32
MB_PER_PIECE = NP // MB   # 8 key blocks per kt/v piece
SKEW = 3             # software-pipeline skew between S and U matmuls

_NC_CACHE = None
LAST_RESULT = None   # BassKernelResults of the most recent kernel() call


def _build():
    nc = bacc.Bacc("TRN2", target_bir_lowering=False, debug=False)

    xa = nc.dram_tensor("xa", [C, N], F32, kind="ExternalInput").ap()
    xb = nc.dram_tensor("xb", [C, N], F32, kind="ExternalInput").ap()
    # host pre-swizzled: [128, CK*C] with (ki, co*128+j) element order
    wq = nc.dram_tensor("wq_l", [128, CK * C], F32, kind="ExternalInput").ap()
    wk = nc.dram_tensor("wk_l", [128, CK * C], F32, kind="ExternalInput").ap()
    wv = nc.dram_tensor("wv_l", [128, CK * C], F32, kind="ExternalInput").ap()
    bq = nc.dram_tensor("bq_l", [128, CK], F32, kind="ExternalInput").ap()
    bk = nc.dram_tensor("bk_l", [128, CK], F32, kind="ExternalInput").ap()
    bv = nc.dram_tensor("bv_l", [1, C], F32, kind="ExternalInput").ap()
    out = nc.dram_tensor("o", [C, N], F32, kind="ExternalOutput").ap()

    with tile.TileContext(nc) as tc:
        with tc.tile_pool(name="persist", bufs=1) as pp, \
             tc.tile_pool(name="xbs", bufs=4) as xbp, \
             tc.tile_pool(name="os", bufs=4) as op_, \
             tc.tile_pool(name="s_ps", bufs=4, space="PSUM") as sp, \
             tc.tile_pool(name="u_ps", bufs=1, space="PSUM") as up, \
             tc.tile_pool(name="e_sb", bufs=5) as ep, \
             tc.tile_pool(name="acc", bufs=2) as ap_:
            # ---- HAM warmup (emitted first, zero data deps) -----------
            warm_w = pp.tile([128, 128], F32R, tag="warm_w")
            warm_src = pp.tile([128, NT], F32R, tag="warm_src")
            nc.vector.memset(warm_w[:].bitcast(F32), 0.0)
            nc.vector.memset(warm_src[:].bitcast(F32), 0.0)
            warm_ps = sp.tile([128, NT], F32, tag="s")
            N_WARM = 26
            for it in range(N_WARM):
                nc.tensor.matmul(warm_ps[:], warm_w[:], warm_src[:],
                                 start=(it == 0), stop=(it == N_WARM - 1))

            # ---- parameters & inputs, in consumption order ------------
            w_r = {}
            for name in ("wq", "wk", "wv"):
                w_r[name] = pp.tile([128, CK, C], F32R, name=f"{name}_r",
                                    tag=f"{name}_r")
            bq_sb = pp.tile([128, CK], F32, tag="bq_sb")
            bk_sb = pp.tile([128, CK], F32, tag="bk_sb")
            bv_r = pp.tile([1, C], F32R, tag="bv_r")
            xa_pieces = {}
            xa_q = {}
            for ki in range(CK):
                for h in range(2):
                    xa_q[(ki, h)] = pp.tile(
                        [128, NT], F32R, name=f"xaq_{ki}_{h}", tag=f"xaq_{ki}_{h}")
            for pc in range(1, 4):
                for ki in range(CK):
                    xa_pieces[(ki, pc)] = pp.tile(
                        [128, NP], F32R, name=f"xa_{ki}_{pc}", tag=f"xa_{ki}_{pc}")

            def xa_rhs(ki, nt):
                # 512-wide rhs slice of xa for QT tile nt
                if nt < 2:
                    return xa_q[(ki, nt)][:]
                piece = xa_pieces[(ki, nt // 2)]
                return piece[:, (nt % 2) * NT:((nt % 2) + 1) * NT]

            def xa_mb(ki, mb):
                # 128-wide lhsT slice of xa for V block mb
                if mb < MB_PER_PIECE:
                    t = xa_q[(ki, mb // 4)]
                    return t[:, (mb % 4) * 128:((mb % 4) + 1) * 128]
                piece = xa_pieces[(ki, mb // MB_PER_PIECE)]
                off = (mb % MB_PER_PIECE) * 128
                return piece[:, off:off + 128]
            xb_pieces = {}
            for pc in range(4):
                for ki in range(CK):
                    xb_pieces[(ki, pc)] = xbp.tile(
                        [128, NP], F32R, name=f"xb_{ki}_{pc}", tag="xb")

            def dma_x(pieces, src, ki, pc):
                nc.sync.dma_start(
                    pieces[(ki, pc)][:],
                    src[ki * 128:(ki + 1) * 128,
                        pc * NP:(pc + 1) * NP].bitcast(F32R))

            nc.sync.dma_start(w_r["wq"][:].rearrange("p k m -> p (k m)"),
                              wq.bitcast(F32R))
            nc.sync.dma_start(bq_sb[:], bq[:])
            for h in range(2):
                for ki in range(CK):
                    nc.sync.dma_start(
                        xa_q[(ki, h)][:],
                        xa[ki * 128:(ki + 1) * 128,
                           h * NT:(h + 1) * NT].bitcast(F32R))
            nc.sync.dma_start(w_r["wv"][:].rearrange("p k m -> p (k m)"),
                              wv.bitcast(F32R))
            nc.sync.dma_start(bv_r[:], bv.bitcast(F32R))
            nc.sync.dma_start(w_r["wk"][:].rearrange("p k m -> p (k m)"),
                              wk.bitcast(F32R))
            nc.sync.dma_start(bk_sb[:], bk[:])
            dma_x(xb_pieces, xb, 0, 0)
            dma_x(xb_pieces, xb, 1, 0)
            for pc in range(1, 4):
                dma_x(xa_pieces, xa, 0, pc)
                dma_x(xa_pieces, xa, 1, pc)
                dma_x(xb_pieces, xb, 0, pc)
                dma_x(xb_pieces, xb, 1, pc)

            ones_f = pp.tile([128, 128], F32, tag="ones_f")
            nc.vector.memset(ones_f[:], 1.0)
            ones_r = pp.tile([128, 128], F32R, tag="ones_r")
            nc.vector.tensor_copy(ones_r[:], ones_f[:])

            # bv replicated to all partitions once (K=1 ones matmul), so the
            # per-block V bias is a DVE add instead of an extra tiny matmul
            bv_ps = sp.tile([128, NT], F32, tag="s")
            nc.tensor.matmul(bv_ps[:, 0:C], ones_r[0:1, :], bv_r[:],
                             start=True, stop=True)
            bv_rep = pp.tile([128, CK, 128], F32, tag="bv_rep")
            nc.vector.tensor_copy(
                bv_rep[:], bv_ps[:, 0:C].rearrange("p (c j) -> p c j", c=CK))

            # per-pair piece tiles for QT / KT / V
            qt_p = [pp.tile([128, CK, NP], F32R, name=f"qt_{p}", tag=f"qt_{p}")
                    for p in range(N_PAIR)]
            kt_p = [pp.tile([128, CK, NP], F32R, name=f"kt_{p}", tag=f"kt_{p}")
                    for p in range(N_PAIR)]
            v_p = [pp.tile([128, CK, NP], F32R, name=f"v_{p}", tag=f"v_{p}")
                   for p in range(N_PAIR)]

            # ---- projection emitters ---------------------------------
            def emit_qtkt(dst_p, w_t, b_sb, pieces, co, nt):
                ps = sp.tile([128, NT], F32, tag="s")
                half = ps[:]
                for ki in range(CK):
                    if pieces is xa_pieces:
                        rhs = xa_rhs(ki, nt)
                    else:
                        piece = pieces[(ki, nt // 2)]
                        rhs = piece[:, (nt % 2) * NT:((nt % 2) + 1) * NT]
                    nc.tensor.matmul(
                        half, w_t[:, ki, co * 128:(co + 1) * 128],
                        rhs, start=(ki == 0), stop=(ki == CK - 1))
                nc.vector.tensor_scalar_add(
                    dst_p[nt // 2][:, co, (nt % 2) * NT:((nt % 2) + 1) * NT],
                    half, b_sb[:, co:co + 1])

            def emit_v(mb):
                ps = sp.tile([128, NT], F32, tag="s")
                half = ps[:, 0:C]
                for ki in range(CK):
                    nc.tensor.matmul(
                        half, xa_mb(ki, mb), w_r["wv"][:, ki, :],
                        start=(ki == 0), stop=(ki == CK - 1))
                off = (mb % MB_PER_PIECE) * 128
                nc.vector.tensor_add(
                    v_p[mb // MB_PER_PIECE][:, :, off:off + 128],
                    half.rearrange("p (c j) -> p c j", c=CK), bv_rep[:])

            # ---- attention emitters ----------------------------------
            pair_state = {}

            def attn_begin(pc):
                pair_state[pc] = {
                    "u": [up.tile([128, NP], F32, name=f"u_{pc}_{co}",
                                  tag=f"u{co}") for co in range(CK)],
                    "dacc": ap_.tile([128, NP], F32R, name=f"dacc_{pc}",
                                     tag="dacc"),
                    "e": {},
                }

            def attn_step(pc, step):
                st = pair_state[pc]
                if step < N_MB:
                    mb = step
                    s_h = [sp.tile([128, NT], F32, name=f"s_h{ho}", tag="s")
                           for ho in range(2)]
                    ktp = kt_p[mb // MB_PER_PIECE]
                    off = (mb % MB_PER_PIECE) * 128
                    for co in range(CK):
                        for ho in range(2):
                            nc.tensor.matmul(
                                s_h[ho][:],
                                ktp[:, co, off:off + 128],
                                qt_p[pc][:, co, ho * NT:(ho + 1) * NT],
                                start=(co == 0), stop=(co == CK - 1))
                    e_r = ep.tile([128, NP], F32R, tag="e")
                    for ho in range(2):
                        nc.scalar.activation(
                            e_r[:, ho * NT:(ho + 1) * NT], s_h[ho][:], AF.Exp)
                    st["e"][mb] = e_r
                if step >= SKEW:
                    mb = step - SKEW
                    e_r = st["e"].pop(mb)
                    vp = v_p[mb // MB_PER_PIECE]
                    off = (mb % MB_PER_PIECE) * 128
                    for co in range(CK):
                        for ho in range(2):
                            nc.tensor.matmul(
                                st["u"][co][:, ho * NT:(ho + 1) * NT],
                                vp[:, co, off:off + 128],
                                e_r[:, ho * NT:(ho + 1) * NT],
                                start=(mb == 0), stop=(mb == N_MB - 1))
                    if mb == 0:
                        nc.vector.tensor_copy(st["dacc"][:], e_r[:])
                    else:
                        nc.vector.tensor_add(st["dacc"][:], st["dacc"][:], e_r[:])

            def attn_end(pc):
                # per-512-half normalize + store so the output DMA of one half
                # overlaps the multiply of the next (shrinks the serial tail)
                st = pair_state.pop(pc)
                dinv = ap_.tile([128, NP], F32, name=f"dinv_{pc}", tag="dinv")
                for ho in range(2):
                    d_ps = sp.tile([128, NT], F32, name=f"d_{ho}", tag="s")
                    nc.tensor.matmul(d_ps[:], ones_r[:],
                                     st["dacc"][:, ho * NT:(ho + 1) * NT],
                                     start=True, stop=True)
                    nc.vector.reciprocal_approx_fast(
                        dinv[:, ho * NT:(ho + 1) * NT], d_ps[:])
                for co in range(CK):
                    for ho in range(2):
                        o_sb = op_.tile([128, NT], F32, tag="o_sb")
                        nc.vector.tensor_mul(
                            o_sb[:], st["u"][co][:, ho * NT:(ho + 1) * NT],
                            dinv[:, ho * NT:(ho + 1) * NT])
                        nc.sync.dma_start(
                            out[co * 128:(co + 1) * 128,
                                pc * NP + ho * NT:pc * NP + (ho + 1) * NT],
                            o_sb[:])

            # ---- emission schedule -----------------------------------
            # Most projection work is distributed just-in-time through pair
            # 0's key-block stream: the PE then always has dense 512-free
            # matmul work and the HAM clock gate never re-throttles (the
            # standalone projection phase ran at 45% duty -> K=4/8 for ~48us
            # in v5). Only the piece-0 prerequisites run up front.
            def emit_qt(co, nt):
                emit_qtkt(qt_p, w_r["wq"], bq_sb, xa_pieces, co, nt)

            def emit_kt(co, nt):
                emit_qtkt(kt_p, w_r["wk"], bk_sb, xb_pieces, co, nt)

            # prerequisites for pair 0, step 0..5
            for co in range(CK):
                for nto in range(2):
                    emit_qt(co, nto)
            for mb in range(8):
                emit_v(mb)
            for co in range(CK):
                for nto in range(2):
                    emit_kt(co, nto)

            # just-in-time jobs sprinkled through pair 0's steps
            extra = {}

            def add_extra(step, fn):
                extra.setdefault(step, []).append(fn)

            for p in range(1, 4):           # kt piece p before S reads it
                for i, (co, nto) in enumerate(
                        ((0, 0), (0, 1), (1, 0), (1, 1))):
                    add_extra(p * MB_PER_PIECE - 6 + i,
                              lambda co=co, nt=2 * p + nto: emit_kt(co, nt))
            for mb in range(8, N_MB):       # v block 4 steps ahead of its U
                add_extra(mb - 4, lambda mb=mb: emit_v(mb))
            for p in range(1, 4):           # qt pieces 1-3 anywhere in pair 0
                for i, (co, nto) in enumerate(
                        ((0, 0), (0, 1), (1, 0), (1, 1))):
                    add_extra(2 + p * 6 + i,
                              lambda co=co, nt=2 * p + nto: emit_qt(co, nt))

            attn_begin(0)
            for step in range(N_MB + SKEW):
                attn_step(0, step)
                for fn in extra.pop(step, ()):
                    fn()
            for pc in range(1, N_PAIR):
                attn_begin(pc)
                for step in range(N_MB + SKEW):
                    attn_step(pc, step)
                    if step == 2:
                        attn_end(pc - 1)
            attn_end(N_PAIR - 1)
    nc.compile()
    return nc


def _get_nc():
    global _NC_CACHE
    if _NC_CACHE is None:
        _NC_CACHE = _build()
    return _NC_CACHE


def _w_layout(w):
    # lhsT chunks: w_l[p, ki*C + m] = W.T[ki*128 + p, m]
    wt = np.ascontiguousarray(np.asarray(w, np.float32).T)      # [C_in, C_out]
    return np.ascontiguousarray(
        wt.reshape(CK, 128, C).transpose(1, 0, 2).reshape(128, CK * C))


def kernel(x1, x2, Wq, bq, Wk, bk, Wv, bv):
    global LAST_RESULT
    x1 = np.asarray(x1, dtype=np.float32)
    x2 = np.asarray(x2, dtype=np.float32)
    shared = {
        "wq_l": _w_layout(Wq),
        "wk_l": _w_layout(Wk),
        "wv_l": _w_layout(Wv),
        "bq_l": np.ascontiguousarray(
            np.asarray(bq, np.float32).reshape(CK, 128).T),
        "bk_l": np.ascontiguousarray(
            np.asarray(bk, np.float32).reshape(CK, 128).T),
        "bv_l": np.asarray(bv, np.float32).reshape(1, C),
    }
    in_maps = []
    for core in range(8):
        b, s = core % B, core // B
        xs, xo = (x1, x2) if s == 0 else (x2, x1)
        in_maps.append({
            "xa": np.ascontiguousarray(xs[b].reshape(C, N)),
            "xb": np.ascontiguousarray(xo[b].reshape(C, N)),
            **shared,
        })
    nc = _get_nc()
    res = run_bass_kernel_spmd(nc, in_maps, list(range(8)))
    LAST_RESULT = res
    x1_out = np.stack([res.results[b]["o"].reshape(C, H, W) for b in range(B)])
    x2_out = np.stack([res.results[B + b]["o"].reshape(C, H, W) for b in range(B)])
    return (x1_out, x2_out)
